# revision 3
# baseline (speedup 1.0000x reference)
"""LMU (Legendre Memory Unit) Trainium2 Bass kernel.

Full-input contract: kernel(**inputs) takes the unsharded inputs from
setup_inputs() and returns the full (64, 2048, 512) output.

Algorithm: the per-step LMU update collapses (by substituting u and m')
into one affine recurrence over z = [h(512); m(256)]:

    m' = m @ (Ad^T + me BT)            + v * BT + pc_m
    h' = tanh(h Whh' + m Wmh + pc_h),  v = h . he   (scalar per batch)

with Whh' = hk + he (BT mk), Wmh = (Ad^T + me BT... folded) mk-ish and
pc = x @ Wx precomputed on-device by a dense GEMM.  Sharding:
data-parallel batch 64 -> 8 cores x 8; the scan runs T sequential
steps with batch N=8 as the matmul moving dim.

Per-step tensor-engine work (the bottleneck is LDWEIGHTS streaming of
the weight blocks, ~53ns per bf16 [128,128] tile with FWL):
  - 4x fp32 [128,128] MMs for the marginally stable m->m block (exact,
    replaces the previous 12-MM bf16 hi/lo scheme)
  - 8x bf16 MMs m->h, 16x bf16 MMs h->h
  - 4x [128,1] column MMs computing v = h.he (near-free weight loads)
  - 2x K=1 outer-product MMs adding v*BT into m
PSUM: h-cols and m-cols live in separate parity-alternating banks so
the m-finalize (DVE) can run while PE still streams h matmuls, and the
next step's first MM never waits on a same-bank DVE read.

Output is transposed on-chip (PE transpose) to [B, T, H] so the host
does no gather; x is relaid out host-side to [k2,128,T,B] so every DMA
moves contiguous 512B descriptors.
"""

import os
import numpy as np
from contextlib import ExitStack

import concourse.bass as bass
import concourse.bacc as bacc
import concourse.tile as tile
import concourse.mybir as mybir
from concourse.bass_utils import run_bass_kernel_spmd

F32 = mybir.dt.float32
BF16 = mybir.dt.bfloat16

B = 8          # batch per core
NCORES = 8
D = 256        # input dim
H = 512        # hidden units
MO = 256       # memory order
Z = H + MO
HT = H // 128  # 4 h tiles
MT2 = MO // 128  # 2 m tiles

_cache = {}


def _build(T: int, ct: int = 32, hints: bool = True):
    """Build the per-core Bass program for sequence length T."""
    assert T % (2 * ct) == 0
    assert ct % 32 == 0
    nc = bacc.Bacc("TRN2", target_bir_lowering=False, debug=False)

    x_d = nc.dram_tensor("x", [2, 128, T, B], BF16, kind="ExternalInput")
    whh_d = nc.dram_tensor("Whh", [H, H], BF16, kind="ExternalInput")
    wmh_d = nc.dram_tensor("Wmh", [MO, H], BF16, kind="ExternalInput")
    wati_d = nc.dram_tensor("Wati", [MO, MO], F32, kind="ExternalInput")
    whe_d = nc.dram_tensor("Whe", [H, 1], BF16, kind="ExternalInput")
    wbt_d = nc.dram_tensor("Wbt", [1, MO], BF16, kind="ExternalInput")
    wx_d = nc.dram_tensor("Wx", [D, Z], BF16, kind="ExternalInput")
    ident_d = nc.dram_tensor("Ident", [128, 128], BF16, kind="ExternalInput")
    out_d = nc.dram_tensor("out", [B, T, H], BF16, kind="ExternalOutput")

    with tile.TileContext(nc) as tc, ExitStack() as ctx:
        const = ctx.enter_context(tc.tile_pool(name="const", bufs=1))
        # weight tiles, kt-major: tile (kt, mt) at col (kt*nmt + mt)*128
        w_hh = const.tile([128, HT * HT * 128], BF16)
        w_mh = const.tile([128, MT2 * HT * 128], BF16)
        w_ati = const.tile([128, MT2 * MT2 * 128], F32)
        w_he = const.tile([128, HT], BF16)          # col kt = he chunk kt
        w_bt = const.tile([1, MO], BF16)
        wx_sb = const.tile([128, 2 * Z], BF16)      # (k2, mt): col k2*Z + mt*128
        ident = const.tile([128, 128], BF16)

        # xbuf: [128, (k2, t, b)]; pcbuf: [128, (t, mt6, b)] f32
        xbuf = [const.tile([128, 2 * ct * B], BF16, name=f"x{i}", tag=f"x{i}")
                for i in range(2)]
        pcbuf = [const.tile([128, ct * 48], F32, name=f"pc{i}", tag=f"pc{i}")
                 for i in range(2)]
        # hbuf: [128, (mt, b, t2)] bf16
        hbuf = [const.tile([128, HT * B * ct], BF16, name=f"h{i}", tag=f"h{i}")
                for i in range(2)]
        # stacked step state: cols 0:32 = h-pre (mt,b), cols 32:48 = m (k2,b)
        sst = [const.tile([128, 48], F32, name=f"s{i}", tag=f"s{i}")
               for i in range(2)]
        mh_sb = [const.tile([128, 2 * B], BF16, name=f"mh{i}", tag=f"mh{i}")
                 for i in range(2)]
        vbuf = [const.tile([1, B], BF16, name=f"v{i}", tag=f"v{i}")
                for i in range(2)]
        obuf = [const.tile([128, H], BF16, name=f"o{i}", tag=f"o{i}")
                for i in range(2)]
        zs32 = const.tile([128, 48], F32)

        ps_h_pool = ctx.enter_context(tc.tile_pool(name="psh", bufs=2, space="PSUM"))
        ps_m_pool = ctx.enter_context(tc.tile_pool(name="psm", bufs=2, space="PSUM"))
        ps_v_pool = ctx.enter_context(tc.tile_pool(name="psv", bufs=1, space="PSUM"))
        pcps_pool = ctx.enter_context(tc.tile_pool(name="pcps", bufs=2, space="PSUM"))
        psT_pool = ctx.enter_context(tc.tile_pool(name="psT", bufs=1, space="PSUM"))

        # --- prologue: weights + state init ---
        for kt in range(HT):
            nc.sync.dma_start(w_hh[:, kt * HT * 128:(kt + 1) * HT * 128],
                              whh_d.ap()[kt * 128:(kt + 1) * 128, :])
        for kt in range(MT2):
            nc.sync.dma_start(w_mh[:, kt * HT * 128:(kt + 1) * HT * 128],
                              wmh_d.ap()[kt * 128:(kt + 1) * 128, :])
            nc.sync.dma_start(w_ati[:, kt * MT2 * 128:(kt + 1) * MT2 * 128],
                              wati_d.ap()[kt * 128:(kt + 1) * 128, :])
        nc.sync.dma_start(
            w_he[:].rearrange("p (k one) -> p k one", one=1),
            whe_d.ap().rearrange("(k p) one -> p k one", p=128))
        nc.sync.dma_start(w_bt[:], wbt_d.ap())
        for k2 in range(2):
            nc.sync.dma_start(wx_sb[:, k2 * Z:(k2 + 1) * Z],
                              wx_d.ap()[k2 * 128:(k2 + 1) * 128, :])
        nc.sync.dma_start(ident[:], ident_d.ap())

        nc.vector.memset(zs32[:], 0.0)
        nc.vector.memset(sst[1][:], 0.0)                      # h/m_{-1} = 0
        nc.vector.tensor_scalar_add(mh_sb[1][:], zs32[:, 0:2 * B], 0.0)
        hzero = hbuf[1][:].rearrange("p (m b t2) -> p m b t2", m=HT, b=B, t2=ct)
        nc.vector.tensor_scalar_add(
            hzero[:, :, :, ct - 1],
            zs32[:, 0:HT * B].rearrange("p (m b) -> p m b", m=HT, b=B), 0.0)

        def dma_x(xb, toff):
            for k2 in range(2):
                nc.sync.dma_start(xb[:, k2 * ct * B:(k2 + 1) * ct * B],
                                  x_d.ap()[k2, :, bass.ds(toff, ct), :])

        def pc_gemm(xb, pcb):
            xv = xb[:].rearrange("p (k t b) -> p k t b", k=2, t=ct, b=B)
            pcv = pcb[:].rearrange("p (t m b) -> p t m b", t=ct, m=6, b=B)
            for mt in range(6):
                ps = pcps_pool.tile([128, ct * B], F32, name="pcp", tag="pcps")
                for k2 in range(2):
                    nc.tensor.matmul(
                        ps[:],
                        wx_sb[:, k2 * Z + mt * 128: k2 * Z + (mt + 1) * 128],
                        xv[:, k2, :, :],
                        start=(k2 == 0), stop=(k2 == 1))
                nc.vector.tensor_scalar_add(
                    pcv[:, :, mt, :],
                    ps[:].rearrange("p (t b) -> p t b", t=ct, b=B), 0.0)

        def scan_chunk(hb, hb_prev, pcb, par0):
            pcv = pcb[:].rearrange("p (t m b) -> p t m b", t=ct, m=6, b=B)
            hv = hb[:].rearrange("p (m b t2) -> p m b t2", m=HT, b=B, t2=ct)
            hv_prev = hb_prev[:].rearrange("p (m b t2) -> p m b t2", m=HT, b=B, t2=ct)
            for t in range(ct):
                par = (par0 + t) % 2
                tp = (t - 1) % ct
                hin = hv_prev if t == 0 else hv
                s_prev, s_cur = sst[1 - par], sst[par]
                mh_prev = mh_sb[1 - par]
                psh = ps_h_pool.tile([128, HT * B], F32, name="psh", tag="psh")
                psm = ps_m_pool.tile([128, MT2 * B], F32, name="psm", tag="psm")
                psv = ps_v_pool.tile([1, B], F32, name="psv", tag="psv")
                # 1) m->m: exact fp32 (marginally stable feedback loop)
                for kt in range(MT2):
                    for mt in range(MT2):
                        nc.tensor.matmul(
                            psm[:, mt * B:(mt + 1) * B],
                            w_ati[:, (kt * MT2 + mt) * 128:(kt * MT2 + mt + 1) * 128],
                            s_prev[:, 32 + kt * B: 32 + (kt + 1) * B],
                            start=(kt == 0 and mt == 0), stop=False,
                            skip_group_check=True)
                # 2) m->h (bf16, rhs = bf16 m)
                for kt in range(MT2):
                    for mt in range(HT):
                        nc.tensor.matmul(
                            psh[:, mt * B:(mt + 1) * B],
                            w_mh[:, (kt * HT + mt) * 128:(kt * HT + mt + 1) * 128],
                            mh_prev[:, kt * B:(kt + 1) * B],
                            start=(kt == 0 and mt == 0), stop=False,
                            skip_group_check=True)
                # 3) v = h . he (near-free [128,1] weight loads)
                for kt in range(HT):
                    nc.tensor.matmul(
                        psv[:], w_he[:, kt:kt + 1], hin[:, kt, :, tp],
                        start=(kt == 0), stop=(kt == HT - 1),
                        skip_group_check=True)
                # 4) h->h first half
                for kt in range(0, 2):
                    for mt in range(HT):
                        nc.tensor.matmul(
                            psh[:, mt * B:(mt + 1) * B],
                            w_hh[:, (kt * HT + mt) * 128:(kt * HT + mt + 1) * 128],
                            hin[:, kt, :, tp],
                            start=False, stop=False, skip_group_check=True)
                # v -> SBUF bf16 (for the outer-product rhs)
                nc.vector.tensor_copy(vbuf[par][:], psv[:])
                # 5) outer product v*BT into m-cols (K=1)
                for mt in range(MT2):
                    nc.tensor.matmul(
                        psm[:, mt * B:(mt + 1) * B],
                        w_bt[0:1, mt * 128:(mt + 1) * 128],
                        vbuf[par][:],
                        start=False, stop=(mt == MT2 - 1),
                        skip_group_check=True)
                # m' = psum_m + pc_m  (fires while PE streams the h tail)
                nc.vector.tensor_add(
                    s_cur[:, 32:48].rearrange("p (k b) -> p k b", k=MT2, b=B),
                    psm[:].rearrange("p (k b) -> p k b", k=MT2, b=B),
                    pcv[:, t, 4:6, :])
                nc.vector.tensor_copy(mh_sb[par][:], s_cur[:, 32:48])
                # 6) h->h second half
                for kt in range(2, HT):
                    for mt in range(HT):
                        nc.tensor.matmul(
                            psh[:, mt * B:(mt + 1) * B],
                            w_hh[:, (kt * HT + mt) * 128:(kt * HT + mt + 1) * 128],
                            hin[:, kt, :, tp],
                            start=False,
                            stop=(kt == HT - 1 and mt == HT - 1),
                            skip_group_check=True)
                # h' = tanh(psum_h + pc_h)
                nc.vector.tensor_add(
                    s_cur[:, 0:32].rearrange("p (m b) -> p m b", m=HT, b=B),
                    psh[:].rearrange("p (m b) -> p m b", m=HT, b=B),
                    pcv[:, t, 0:4, :])
                nc.scalar.activation(
                    hv[:, :, :, t],
                    s_cur[:, 0:32].rearrange("p (m b) -> p m b", m=HT, b=B),
                    mybir.ActivationFunctionType.Tanh)

        def dma_out(hb, toff):
            # on-chip transpose [h-chunk, (b,t2)] -> [(b,t2), h] then DMA
            nb = 128 // ct            # batch elements per 128-col transpose group
            for g in range(B // nb):
                psT = psT_pool.tile([128, H], BF16, name="psT", tag="psT")
                for mt in range(HT):
                    nc.tensor.transpose(
                        psT[:, mt * 128:(mt + 1) * 128],
                        hb[:, mt * B * ct + g * nb * ct: mt * B * ct + (g + 1) * nb * ct],
                        ident[:])
                ob = obuf[g % 2]
                nc.vector.tensor_copy(ob[:], psT[:])
                nc.sync.dma_start(
                    out_d.ap()[bass.ds(g * nb, nb), bass.ds(toff, ct), :], ob[:])

        def body(toff):
            dma_x(xbuf[0], toff)
            pc_gemm(xbuf[0], pcbuf[0])
            dma_x(xbuf[1], toff + ct)
            scan_chunk(hbuf[0], hbuf[1], pcbuf[0], 0)
            dma_out(hbuf[0], toff)
            pc_gemm(xbuf[1], pcbuf[1])
            scan_chunk(hbuf[1], hbuf[0], pcbuf[1], 0)
            dma_out(hbuf[1], toff + ct)

        he = (mybir.EngineType.PE,) if hints else ()
        with tc.For_i(0, T, 2 * ct, hint_engines=he) as toff:
            body(toff)

    nc.compile()
    return nc


def _host_weights(inputs):
    """Fold the LMU weights into the per-block device weights, f64 host math."""
    ie = np.asarray(inputs["input_encoders"], np.float64)    # (256,1)
    he = np.asarray(inputs["hidden_encoders"], np.float64)   # (512,1)
    me = np.asarray(inputs["memory_encoders"], np.float64)   # (256,1)
    ik = np.asarray(inputs["input_kernel"], np.float64)      # (256,512)
    hk = np.asarray(inputs["hidden_kernel"], np.float64)     # (512,512)
    mk = np.asarray(inputs["memory_kernel"], np.float64)     # (256,512)
    AT = np.asarray(inputs["AT"], np.float64)                # (256,256)
    BT = np.asarray(inputs["BT"], np.float64)                # (1,256)
    ATI = AT + np.eye(MO)            # = Ad^T
    g = BT @ mk                      # (1,512)
    Whh = hk + he @ g                                 # h->h
    Wmh = ATI @ mk + me @ g                           # m->h
    Wati = ATI + me @ BT                              # m->m (fp32)
    Wx = np.zeros((D, Z))
    Wx[:, 0:H] = ik + ie @ g
    Wx[:, H:Z] = ie @ BT
    return (Whh.astype(np.float32), Wmh.astype(np.float32),
            Wati.astype(np.float32), he.astype(np.float32),
            BT.astype(np.float32), Wx.astype(np.float32))


def kernel(**inputs):
    import ml_dtypes
    b16 = ml_dtypes.bfloat16

    x = np.ascontiguousarray(np.asarray(inputs["x"], np.float32))
    Bfull, T, _ = x.shape
    Whh, Wmh, Wati, he, BT, Wx = _host_weights(inputs)

    ct = int(os.environ.get("LMU_CT", "32"))
    hints = os.environ.get("LMU_HINTS", "1") == "1"
    key = (T, ct, hints)
    if key not in _cache:
        _cache[key] = _build(T, ct=ct, hints=hints)
    nc = _cache[key]

    # relayout x: [64, T, 256] -> [2, 128, T, 64]  (k2, p, t, b)
    xT = np.ascontiguousarray(
        x.astype(b16).reshape(Bfull, T, 2, 128).transpose(2, 3, 1, 0))

    ident = np.eye(128, dtype=b16)
    per = Bfull // NCORES
    base = {"Whh": Whh.astype(b16), "Wmh": Wmh.astype(b16), "Wati": Wati,
            "Whe": he.astype(b16), "Wbt": BT.astype(b16),
            "Wx": Wx.astype(b16), "Ident": ident}
    in_maps = [
        dict(base, x=np.ascontiguousarray(xT[:, :, :, c * per:(c + 1) * per]))
        for c in range(NCORES)
    ]
    res = run_bass_kernel_spmd(nc, in_maps, core_ids=list(range(NCORES)))
    out = np.concatenate([np.asarray(r["out"]) for r in res.results], axis=0)
    return out.astype(np.float32)
